# revision 1
# baseline (speedup 1.0000x reference)
"""Multi-head attention (B=4, S=2048, E=768, H=12) on 8 NeuronCores.

Sharding: core c handles batch c//2 and head-group c%2 (6 heads = 3 pairs).
Each core computes its heads' attention plus a partial output projection
(E-dim split); partials are summed on the host and bias added there.

Device-side layout (per core):
  - x^T [768, 2048] streamed in; QKV projection done with W as the
    stationary operand so Q^T/K^T come out in [d, s] layout with a head
    PAIR stacked on partition halves (head A -> partitions 0-63,
    head B -> 64-127).
  - Scores computed transposed (ST[k, q] = K Q^T) with row-tiled K=64
    matmuls (2 heads concurrently in the PE array).
  - exp on the scalar engine (PSUM -> SBUF, fused 1/sqrt(64) scale),
    no max subtraction (scores are ~N(0,1); exp cannot overflow fp32).
  - P^T stays in [k, q] layout so A@V runs as V^T P^T with full K=128
    contraction; V carries an extra ones column so the softmax
    denominator accumulates in PSUM row 64 for free.
  - Normalize via DVE reciprocal + a ones-stationary PE broadcast
    matmul + DVE mul.
  - Output projection accumulated over the 3 pairs.

Engine budget (HW-measured via microbenchmarks; this environment caps
the PE clock at ~1.2-1.3 GHz effective, session-varying, and provides
no NTFF tracing):
  - PE is the critical engine: ~350us/core of streamed matmul columns
    (ST 78 + AV 152 + QKV 85 + proj 28 + bc 9).  Cost law: a matmul
    costs N cycles regardless of M/K, so only genuinely concurrent
    row/col-tiled pairs (disjoint 32-row/col groups, adjacent in the
    PE queue) beat serial streaming.
  - ACT (softmax exp) ~190us and DVE ~110us ride underneath.
QKV-projection and output-projection matmuls are emitted as fine-grained
filler units inside the attention kt-loop to keep every engine busy.

The two 64-row score matmuls of each head pair auto-derive
tile_position (0,0)/(64,0) and co-stream on disjoint row groups (2x,
HW-verified) -- but only when adjacent in the PE queue, so the second
one is emitted under tc.high_priority() to stop earlier-emitted AV
matmuls from being heap-scheduled between them.

Score/AV/proj matmuls run in float32r (single-pass fp32 mode, same PE
rate as bf16 here); accumulation is fp32 in PSUM.  x and W_qkv ship to
the device as bfloat16 (halved input DMA + FWL fast weight loads for
the all-bf16 QKV matmuls; won 3/4 interleaved A/B rounds by 20-60us),
and the exp output P and V tiles are bf16 (halves their SBUF
footprint, funds a deeper pt pool).  Total rel err ~5.6e-3 vs the
2e-2 gate.  Column-tiled matmul outputs at
nonzero partition base fail walrus ISA checks (s3d3_mm_valid_dst_
partition) in this toolchain, and DMA access patterns reject stride-0
partition dims, so the AV+denominator block (1024 cyc/kt-iter) and the
PE recip broadcasts are at their floor.
"""

import numpy as np

EMBED = 768
HEADS = 12
HD = 64
B = 4
S = 2048
N_CORES = 8
HPC = 6      # heads per core
PAIRS = 3    # head pairs per core
EKT = EMBED // 128   # 6 contraction tiles over E
SKT = S // 128       # 16 key tiles
QC = 512             # q-chunk (matmul free dim)
NQC = S // QC        # 4 q-chunks

BF16_IN = True   # ship x / W_qkv to the device as bfloat16
_CACHE = {}
LAST_RESULTS = None  # stashed BassKernelResults for test harnesses


def _build(repeat=1, ablate=(), unroll=1, dma_bc=False, dma_order=False,
           lag=4, ptp_bufs=8, big_bufs=8, bf16_in=BF16_IN, qk16=True,
           proj16=True):
    # repeat>1 wraps the body in a hardware loop -- used only by timing
    # harnesses to amplify exec time above host-side dispatch noise.
    # unroll: bodies emitted per For_i back-edge (the back-edge is an
    # all-engine sync, so unroll>1 lets consecutive bodies pipeline).
    # ablate: timing-only ablations ("tinyexp", "tinyav") that produce
    # garbage output but isolate engine costs for bottleneck hunting.
    # dma_bc: broadcast the reciprocal via SBUF->SBUF DMA instead of a
    # PE matmul -- DEAD on this toolchain (DMA APs reject stride-0
    # partition dims), kept for reference.
    # dma_order: issue the input DMAs in first-consumer order (wq/wk of
    # pair 0, then the c=0 x^T chunks) to shrink the PE start bubble.
    import contextlib
    import concourse.bacc as bacc
    import concourse.tile as tile
    from concourse import mybir

    F32R = mybir.dt.float32r
    F32 = mybir.dt.float32
    BF16 = mybir.dt.bfloat16
    Exp = mybir.ActivationFunctionType.Exp
    IN = BF16 if bf16_in else F32R

    nc = bacc.Bacc(None, target_bir_lowering=False)

    xt_d = nc.dram_tensor("xt", [EMBED, S], IN, kind="ExternalInput")
    wq_d = nc.dram_tensor("wq", [EMBED, HPC * HD], IN, kind="ExternalInput")
    wk_d = nc.dram_tensor("wk", [EMBED, HPC * HD], IN, kind="ExternalInput")
    wv_d = nc.dram_tensor("wv", [EMBED, HPC * HD], IN, kind="ExternalInput")
    wp_d = nc.dram_tensor("wp", [HPC * HD, EMBED],
                          BF16 if proj16 else F32R, kind="ExternalInput")
    o_d = nc.dram_tensor("o", [EMBED, S], F32, kind="ExternalOutput")

    with tile.TileContext(nc) as tc:
        with tc.tile_pool(name="w", bufs=1) as wpool, \
             tc.tile_pool(name="big", bufs=big_bufs) as big, \
             tc.tile_pool(name="v", bufs=1) as vpool, \
             tc.tile_pool(name="pt", bufs=ptp_bufs) as ptp, \
             tc.tile_pool(name="nrm", bufs=2) as nrm, \
             tc.tile_pool(name="mm_ps", bufs=2, space="PSUM") as mm_ps, \
             tc.tile_pool(name="st_ps", bufs=2, space="PSUM") as st_ps, \
             tc.tile_pool(name="av_ps", bufs=1, space="PSUM") as av_ps:

            # ---- resident inputs (weights first, x^T split per
            #      (E-tile, seq-chunk) so early matmuls only wait on the
            #      pieces they actually read) ----
            wq = wpool.tile([128, EKT, HPC * HD], IN)
            wk = wpool.tile([128, EKT, HPC * HD], IN)
            wv = wpool.tile([128, EKT, HPC * HD], IN)
            xts = [wpool.tile([128, S], IN, name=f"xt{e}", tag=f"xt{e}")
                   for e in range(EKT)]
            wp = wpool.tile([128, PAIRS, EMBED],
                            BF16 if proj16 else F32R)

            def dma_wqk(p):
                blk = slice(p * 128, (p + 1) * 128)
                nc.sync.dma_start(
                    wq[:, :, blk],
                    wq_d.rearrange("(t p) m -> p t m", p=128)[:, :, blk])
                nc.sync.dma_start(
                    wk[:, :, blk],
                    wk_d.rearrange("(t p) m -> p t m", p=128)[:, :, blk])

            def dma_xt(c):
                for e in range(EKT):
                    nc.sync.dma_start(
                        xts[e][:, c * QC:(c + 1) * QC],
                        xt_d[e * 128:(e + 1) * 128, c * QC:(c + 1) * QC])

            if dma_order:
                # first-consumer order: the prologue's first matmul is the
                # (pair 0, chunk 0) Q projection, which reads wq pair 0 and
                # the c=0 x^T chunks -- put exactly those first.
                dma_wqk(0)
                dma_xt(0)
                dma_wqk(1)
                dma_wqk(2)
                nc.sync.dma_start(wv[:],
                                  wv_d.rearrange("(t p) m -> p t m", p=128))
                for c in range(1, NQC):
                    dma_xt(c)
            else:
                for p in range(PAIRS):
                    dma_wqk(p)
                nc.sync.dma_start(wv[:],
                                  wv_d.rearrange("(t p) m -> p t m", p=128))
                for c in range(NQC):
                    dma_xt(c)
            nc.sync.dma_start(wp[:], wp_d.rearrange("(t p) e -> p t e", p=128))

            # V in natural [s, d] layout: per s-tile, [head, 65] with a
            # ones column (row 64 of the AV output = softmax denominator)
            v_sb = [vpool.tile([128, HPC, HD + 1], BF16, name=f"v{st}",
                               tag=f"v{st}") for st in range(SKT)]
            ones = vpool.tile([128, 1], F32)
            nc.vector.memset(ones[:], 1.0)
            ones_r = vpool.tile([1, HD], F32R)
            nc.vector.tensor_copy(ones_r[0:1, :], ones[0:1, 0:1].broadcast_to([1, HD]))
            for st in range(SKT):
                nc.vector.tensor_copy(
                    v_sb[st][:, :, HD:HD + 1],
                    ones[:, None, :].broadcast_to([128, HPC, 1]))

            assert repeat % unroll == 0
            rep_ctx = (tc.For_i(0, repeat // unroll, 1) if repeat > unroll
                       else contextlib.nullcontext())
            rep_ctx.__enter__()

            qts = [None] * PAIRS
            kts = [None] * PAIRS
            ons = [None] * PAIRS

            def alloc_qk(p):
                if qts[p] is None:
                    qk_dt = BF16 if qk16 else F32R
                    qts[p] = big.tile([128, S], qk_dt, tag="big", name=f"qt{p}")
                    kts[p] = big.tile([128, S], qk_dt, tag="big", name=f"kt{p}")

            def emit_qk_unit(p, c, which):
                """One projection unit: 6 matmuls + copyback for Q or K,
                pair p, sequence chunk c."""
                alloc_qk(p)
                w_sb, dst = (wq, qts[p]) if which == "q" else (wk, kts[p])
                ps = mm_ps.tile([128, QC], F32, tag="mm")
                for e in range(EKT):
                    nc.tensor.matmul(
                        ps[:], w_sb[:, e, p * 128:(p + 1) * 128],
                        xts[e][:, c * QC:(c + 1) * QC],
                        start=(e == 0), stop=(e == EKT - 1))
                nc.vector.tensor_copy(dst[:, c * QC:(c + 1) * QC], ps[:])

            def emit_v_unit(st):
                """V for s-tile st, all 6 heads."""
                ps = mm_ps.tile([128, HPC * HD], F32, tag="mm")
                for e in range(EKT):
                    nc.tensor.matmul(
                        ps[:], xts[e][:, st * 128:(st + 1) * 128], wv[:, e, :],
                        start=(e == 0), stop=(e == EKT - 1))
                nc.vector.tensor_copy(
                    v_sb[st][:, :, 0:HD],
                    ps[:].rearrange("p (h d) -> p h d", h=HPC))

            def emit_proj_unit(qc, et):
                """Output projection for q-chunk qc, one e-tile."""
                ps = mm_ps.tile([128, QC], F32, tag="mm")
                for p in range(PAIRS):
                    nc.tensor.matmul(
                        ps[:], wp[:, p, et * 128:(et + 1) * 128],
                        ons[p][:, qc * QC:(qc + 1) * QC],
                        start=(p == 0), stop=(p == PAIRS - 1))
                o_sb = nrm.tile([128, QC], F32, tag="o_sb", bufs=2)
                nc.vector.tensor_copy(o_sb[:], ps[:])
                nc.sync.dma_start(
                    o_d[et * 128:(et + 1) * 128, qc * QC:(qc + 1) * QC], o_sb[:])

            def emit_attn_qc(p, qc, fillers):
                """One q-chunk (512) of attention for head pair p.

                `fillers` is a list of zero-arg emission callbacks (QKV or
                proj units for other pairs) spread across the kt loop so
                the PE/DVE queues never hold a long serial run while the
                scalar engine streams exps.
                """
                av = av_ps.tile([65, 2, QC], F32, tag="av")
                AV_LAG = lag  # emit AV(kt-LAG) after ST(kt) so a blocked AV
                #             (av WAR on the previous chunk's normalize)
                #             never starves the exp stream of fresh STs
                pts = {}

                def emit_av(kt):
                    pt = pts.pop(kt)
                    if "tinyav" in ablate:
                        for h in range(2):
                            nc.tensor.matmul(
                                av[:, h, 0:8], v_sb[kt][:, 2 * p + h, :],
                                pt[:, h * QC:h * QC + 8],
                                start=(kt == 0), stop=(kt == SKT - 1))
                        return
                    for h in range(2):
                        nc.tensor.matmul(
                            av[:, h, :], v_sb[kt][:, 2 * p + h, :],
                            pt[:, h * QC:(h + 1) * QC],
                            start=(kt == 0), stop=(kt == SKT - 1))

                for kt in range(SKT):
                    st = st_ps.tile([128, 2 * QC], F32, tag="st")
                    # the second (rows 64-127) score matmul gets a priority
                    # boost so the scheduler keeps the row-tiled pair
                    # adjacent in the PE queue -- adjacent 64-row matmuls
                    # co-stream on disjoint row groups (2x throughput);
                    # an earlier-emitted AV becoming ready mid-pair would
                    # otherwise be heap-popped between them.
                    nc.tensor.matmul(
                        st[:, 0:QC],
                        kts[p][0:64, kt * 128:(kt + 1) * 128],
                        qts[p][0:64, qc * QC:(qc + 1) * QC],
                        start=True, stop=True)
                    with tc.high_priority(offset=200):
                        nc.tensor.matmul(
                            st[:, QC:2 * QC],
                            kts[p][64:128, kt * 128:(kt + 1) * 128],
                            qts[p][64:128, qc * QC:(qc + 1) * QC],
                            start=True, stop=True)
                    pt = ptp.tile([128, 2 * QC], BF16, tag="pt")
                    if "tinyexp" in ablate:
                        nc.scalar.activation(pt[:, 0:8], st[:, 0:8], Exp,
                                             scale=float(HD) ** -0.5)
                    else:
                        nc.scalar.activation(pt[:], st[:], Exp, scale=float(HD) ** -0.5)
                    pts[kt] = pt
                    if kt >= AV_LAG:
                        emit_av(kt - AV_LAG)
                    if kt < len(fillers):
                        fillers[kt]()
                for kt in range(SKT - AV_LAG, SKT):
                    emit_av(kt)
                if ons[p] is None:
                    ons[p] = big.tile([128, S], BF16 if proj16 else F32R,
                                      tag="big", name=f"on{p}")
                # single copy frees the av accumulator; the rest of the
                # normalize chain runs off-PSUM without blocking the next
                # chunk's AV matmuls
                av_sb = nrm.tile([65, 2, QC], F32, tag="av_sb", bufs=2)
                nc.vector.tensor_copy(av_sb[:], av[:])
                recip = nrm.tile([1, 2, QC], F32R, tag="recip", bufs=1)
                with nc.allow_low_precision(reason="f32r recip for PE bcast"):
                    nc.vector.reciprocal(recip[0:1, :, :], av_sb[64:65, :, :])
                for h in range(2):
                    if dma_bc:
                        # partition-broadcast the [1, QC] reciprocal row to
                        # 64 partitions on the DMA engine (stride-0 source)
                        # instead of a ones-stationary PE matmul
                        bc_sb = nrm.tile([64, QC], F32R, tag="bc_sb", bufs=2)
                        nc.sync.dma_start(
                            bc_sb[:],
                            recip[0:1, h, :].broadcast_to([64, QC]))
                        bc = bc_sb
                    else:
                        bc = mm_ps.tile([64, QC], F32, tag="mm")
                        nc.tensor.matmul(bc[:], ones_r[0:1, :],
                                         recip[0:1, h, :],
                                         start=True, stop=True)
                    nc.vector.tensor_mul(
                        ons[p][h * 64:(h + 1) * 64, qc * QC:(qc + 1) * QC],
                        av_sb[0:64, h, :], bc[:])

            # ---- emission schedule ----
            # Correctness constraints on emission order (Tile serializes
            # reads after writes in program order):
            #  - K(p, c) is read by attn(p, qc, kt=4c..4c+3) for EVERY qc,
            #    so all 4 K chunks must be emitted before (or early inside
            #    via fillers) pair p's first q-chunk.
            #  - Q(p, c) is read only by attn(p, qc=c).
            #  - V(s) is read by AV(kt=s), emitted at loop slot s+AV_LAG.
            # Everything not needed immediately rides as a filler unit so
            # the scalar engine's exp stream starts ~5us in.
            def emit_body():
                qts[:] = [None] * PAIRS
                kts[:] = [None] * PAIRS
                ons[:] = [None] * PAIRS

                emit_qk_unit(0, 0, "q")
                emit_qk_unit(0, 0, "k")
                emit_qk_unit(0, 1, "q")
                for s in range(3):
                    emit_v_unit(s)

                qk = lambda p, c, w: (lambda: emit_qk_unit(p, c, w))
                vu = lambda st: (lambda: emit_v_unit(st))
                pj = lambda q, e: (lambda: emit_proj_unit(q, e))

                # Fillers ride in the PE slack of the ACT-bound kt loops.
                # Ordering constraints: K(p, c) before pair p's first chunk
                # reads key-tile 4c; Q(p, c) before chunk (p, qc=c); V(s) no
                # later than slot s + AV_LAG - 1 of (0, 0).
                filler_sched = {
                    (0, 0): [qk(0, 1, "k"), qk(0, 2, "k"), qk(0, 3, "k")]
                            + [vu(s) for s in range(3, 16)],
                    (0, 1): [qk(0, 2, "q"), qk(0, 3, "q"), qk(1, 0, "q"),
                             qk(1, 0, "k"), qk(1, 1, "k"), qk(1, 2, "k"),
                             qk(1, 3, "k")],
                    (0, 2): [qk(1, 1, "q"), qk(1, 2, "q"), qk(1, 3, "q")],
                    (1, 0): [qk(2, 0, "q"), qk(2, 0, "k"), qk(2, 1, "k"),
                             qk(2, 2, "k"), qk(2, 3, "k")],
                    (1, 1): [qk(2, 1, "q"), qk(2, 2, "q"), qk(2, 3, "q")],
                    (2, 1): [pj(0, e) for e in range(EMBED // 128)],
                    (2, 2): [pj(1, e) for e in range(EMBED // 128)],
                    (2, 3): [pj(2, e) for e in range(EMBED // 128)],
                }
                for p in range(PAIRS):
                    for qc in range(NQC):
                        emit_attn_qc(p, qc, filler_sched.get((p, qc), []))
                for et in range(EMBED // 128):
                    emit_proj_unit(NQC - 1, et)

            for _ in range(min(unroll, repeat)):
                emit_body()

            rep_ctx.__exit__(None, None, None)

    nc.compile()
    return nc


def _get_nc():
    if "nc" not in _CACHE:
        _CACHE["nc"] = _build()
    return _CACHE["nc"]


PROJ16 = True    # ship W_proj as bfloat16 too


def in_dt16():
    import ml_dtypes
    return ml_dtypes.bfloat16


def _shard(x, W_qkv, W_proj):
    if BF16_IN:
        in_dt = in_dt16()
    else:
        in_dt = np.float32
    ins = []
    for c in range(N_CORES):
        b, g = divmod(c, 2)
        cols = slice(g * HPC * HD, (g + 1) * HPC * HD)
        ins.append({
            "xt": np.ascontiguousarray(x[b].T).astype(in_dt),
            "wq": np.ascontiguousarray(
                W_qkv[:, 0 * EMBED:1 * EMBED][:, cols]).astype(in_dt),
            "wk": np.ascontiguousarray(
                W_qkv[:, 1 * EMBED:2 * EMBED][:, cols]).astype(in_dt),
            "wv": np.ascontiguousarray(
                W_qkv[:, 2 * EMBED:3 * EMBED][:, cols]).astype(in_dt),
            "wp": np.ascontiguousarray(W_proj[cols, :]).astype(
                np.float32 if not PROJ16 else in_dt16()),
        })
    return ins


def kernel(x, W_qkv, W_proj, b_proj):
    from concourse.bass_utils import run_bass_kernel_spmd
    global LAST_RESULTS

    x = np.asarray(x, dtype=np.float32)
    W_qkv = np.asarray(W_qkv, dtype=np.float32)
    W_proj = np.asarray(W_proj, dtype=np.float32)
    b_proj = np.asarray(b_proj, dtype=np.float32)

    ins = _shard(x, W_qkv, W_proj)

    nc = _get_nc()
    res = run_bass_kernel_spmd(nc, ins, core_ids=list(range(N_CORES)))
    LAST_RESULTS = res

    out = np.empty((B, S, EMBED), np.float32)
    for b in range(B):
        acc = res.results[2 * b]["o"] + res.results[2 * b + 1]["o"]
        out[b] = acc.T + b_proj
    return out



# revision 32
# speedup vs baseline: 1.2845x; 1.2845x over previous
"""Multi-head attention (B=4, S=2048, E=768, H=12) on 8 NeuronCores.

Sharding: core c handles batch c//2 and head-group c%2 (6 heads = 3 pairs).
Each core computes its heads' attention plus a partial output projection
(E-dim split); partials are summed on the host and bias added there.

Device-side layout (per core):
  - x^T [768, 2048] streamed in; QKV projection done with W as the
    stationary operand so Q^T/K^T come out in [d, s] layout with a head
    PAIR stacked on partition halves (head A -> partitions 0-63,
    head B -> 64-127).
  - Scores computed transposed (ST[k, q] = K Q^T) with row-tiled K=64
    matmuls (2 heads concurrently in the PE array).
  - exp on the scalar engine (PSUM -> SBUF, fused 1/sqrt(64) scale),
    no max subtraction (scores are ~N(0,1); exp cannot overflow fp32).
  - A@V (av_qd mode, default): P^T [k, q] 128x128 tiles are the
    STATIONARY; V [k, d] (+ a ones column) streams 65 columns per
    (head, q-subtile) -> out [q, d] with the softmax denominator in
    column 64.  8 matmuls per kt at 65 cols each (520 cyc/kt vs 1024
    for the old V-stationary scheme; stationary reloads hide under the
    streams).  The 4 q-subtile chains of one head share a PSUM bank:
    one accumulation group per 2KB zero region, so only the first
    chain start=True (marks the whole bank pending-zero) and only the
    last stop=True.
  - Normalize in [q, d] layout: DVE reciprocal of column 64 +
    free-dim-broadcast mul (no PE broadcast needed), then a 128x128
    PE transpose (both heads stacked) back to [hd, q] for the
    projection.
  - Output projection accumulated over the 3 pairs.

Engine budget: PE stream cycles = ST 98K + AV 100K + QKV 74K + V 37K +
proj 37K + transpose 6K = 352K cyc; measured ~245-280us/iter in quiet
windows (effective ~1.3-1.6 GHz incl. per-instruction overheads), down
from ~353us for the V-stationary baseline.  ACT (softmax exp, 192
instrs of [128,1024]) ~190us busy and partly on the critical path
(tinyexp ablation: -53us): the span is an ACT/PE co-bound interleave.
QKV-projection and output-projection matmuls are emitted as
fine-grained filler units inside the attention kt-loop to keep every
engine busy; st_hi lifts all score matmuls above fillers/AV in the
Tile ready-heap so a 6-matmul filler unit never starves the exp feed
(~10us, measured).

The two 64-row score matmuls of each head pair auto-derive
tile_position (0,0)/(64,0) and co-stream on disjoint row groups (2x,
HW-verified) -- but only when adjacent in the PE queue, so the second
one is emitted under tc.high_priority() to stop earlier-emitted AV
matmuls from being heap-scheduled between them.

x and W_qkv ship to the device as bfloat16 (halved input DMA + FWL
fast weight loads), and the exp output P and V tiles are bf16.  Total
rel err ~7.7e-3 vs the 2e-2 gate.  Measurement noise on these shared
axon devices is heavily one-sided (external stalls); compare variants
with min-over-samples deltas (bench.py), interleaved in time.
"""

import numpy as np

EMBED = 768
HEADS = 12
HD = 64
B = 4
S = 2048
N_CORES = 8
HPC = 6      # heads per core
PAIRS = 3    # head pairs per core
EKT = EMBED // 128   # 6 contraction tiles over E
SKT = S // 128       # 16 key tiles
QC = 512             # q-chunk (matmul free dim)
NQC = S // QC        # 4 q-chunks

BF16_IN = True   # ship x / W_qkv to the device as bfloat16
_CACHE = {}
LAST_RESULTS = None  # stashed BassKernelResults for test harnesses


def _build(repeat=1, ablate=(), unroll=1, dma_bc=False, dma_order=False,
           lag=4, ptp_bufs=8, big_bufs=8, bf16_in=BF16_IN, qk16=True,
           proj16=True, av_qd=True, split_acc=False, tp_defer=False,
           sched2=False, tail_proj=False, exp2=False, st_hi=800):
    # repeat>1 wraps the body in a hardware loop -- used only by timing
    # harnesses to amplify exec time above host-side dispatch noise.
    # unroll: bodies emitted per For_i back-edge (the back-edge is an
    # all-engine sync, so unroll>1 lets consecutive bodies pipeline).
    # ablate: timing-only ablations ("tinyexp", "tinyav") that produce
    # garbage output but isolate engine costs for bottleneck hunting.
    # dma_bc: broadcast the reciprocal via SBUF->SBUF DMA instead of a
    # PE matmul -- DEAD on this toolchain (DMA APs reject stride-0
    # partition dims), kept for reference.
    # dma_order: issue the input DMAs in first-consumer order (wq/wk of
    # pair 0, then the c=0 x^T chunks) to shrink the PE start bubble.
    import contextlib
    import concourse.bacc as bacc
    import concourse.tile as tile
    from concourse import mybir

    F32R = mybir.dt.float32r
    F32 = mybir.dt.float32
    BF16 = mybir.dt.bfloat16
    Exp = mybir.ActivationFunctionType.Exp
    IN = BF16 if bf16_in else F32R

    nc = bacc.Bacc(None, target_bir_lowering=False)

    xt_d = nc.dram_tensor("xt", [EMBED, S], IN, kind="ExternalInput")
    wq_d = nc.dram_tensor("wq", [EMBED, HPC * HD], IN, kind="ExternalInput")
    wk_d = nc.dram_tensor("wk", [EMBED, HPC * HD], IN, kind="ExternalInput")
    wv_d = nc.dram_tensor("wv", [EMBED, HPC * HD], IN, kind="ExternalInput")
    wp_d = nc.dram_tensor("wp", [HPC * HD, EMBED],
                          BF16 if proj16 else F32R, kind="ExternalInput")
    o_d = nc.dram_tensor("o", [EMBED, S], F32, kind="ExternalOutput")

    with tile.TileContext(nc) as tc:
        with tc.tile_pool(name="w", bufs=1) as wpool, \
             tc.tile_pool(name="big", bufs=big_bufs) as big, \
             tc.tile_pool(name="v", bufs=1) as vpool, \
             tc.tile_pool(name="pt", bufs=ptp_bufs) as ptp, \
             tc.tile_pool(name="nrm", bufs=2) as nrm, \
             tc.tile_pool(name="mm_ps", bufs=2, space="PSUM") as mm_ps, \
             tc.tile_pool(name="st_ps", bufs=(1 if exp2 else 2),
                          space="PSUM") as st_ps, \
             tc.tile_pool(name="av_ps", bufs=1, space="PSUM") as av_ps:

            # ---- resident inputs (weights first, x^T split per
            #      (E-tile, seq-chunk) so early matmuls only wait on the
            #      pieces they actually read) ----
            wq = wpool.tile([128, EKT, HPC * HD], IN)
            wk = wpool.tile([128, EKT, HPC * HD], IN)
            wv = wpool.tile([128, EKT, HPC * HD], IN)
            xts = [wpool.tile([128, S], IN, name=f"xt{e}", tag=f"xt{e}")
                   for e in range(EKT)]
            wp = wpool.tile([128, PAIRS, EMBED],
                            BF16 if proj16 else F32R)

            def dma_wqk(p):
                blk = slice(p * 128, (p + 1) * 128)
                nc.sync.dma_start(
                    wq[:, :, blk],
                    wq_d.rearrange("(t p) m -> p t m", p=128)[:, :, blk])
                nc.sync.dma_start(
                    wk[:, :, blk],
                    wk_d.rearrange("(t p) m -> p t m", p=128)[:, :, blk])

            def dma_xt(c):
                for e in range(EKT):
                    nc.sync.dma_start(
                        xts[e][:, c * QC:(c + 1) * QC],
                        xt_d[e * 128:(e + 1) * 128, c * QC:(c + 1) * QC])

            if dma_order:
                # first-consumer order: the prologue's first matmul is the
                # (pair 0, chunk 0) Q projection, which reads wq pair 0 and
                # the c=0 x^T chunks -- put exactly those first.
                dma_wqk(0)
                dma_xt(0)
                dma_wqk(1)
                dma_wqk(2)
                nc.sync.dma_start(wv[:],
                                  wv_d.rearrange("(t p) m -> p t m", p=128))
                for c in range(1, NQC):
                    dma_xt(c)
            else:
                for p in range(PAIRS):
                    dma_wqk(p)
                nc.sync.dma_start(wv[:],
                                  wv_d.rearrange("(t p) m -> p t m", p=128))
                for c in range(NQC):
                    dma_xt(c)
            nc.sync.dma_start(wp[:], wp_d.rearrange("(t p) e -> p t e", p=128))

            # V in natural [s, d] layout: per s-tile, [head, 65] with a
            # ones column (row 64 of the AV output = softmax denominator)
            v_sb = [vpool.tile([128, HPC, HD + 1], BF16, name=f"v{st}",
                               tag=f"v{st}") for st in range(SKT)]
            ones = vpool.tile([128, 1], F32)
            nc.vector.memset(ones[:], 1.0)
            if not av_qd:
                ones_r = vpool.tile([1, HD], F32R)
                nc.vector.tensor_copy(
                    ones_r[0:1, :], ones[0:1, 0:1].broadcast_to([1, HD]))
            if av_qd:
                from concourse.masks import make_identity
                ident = vpool.tile([128, 128], BF16)
                make_identity(nc, ident[:])
            for st in range(SKT):
                nc.vector.tensor_copy(
                    v_sb[st][:, :, HD:HD + 1],
                    ones[:, None, :].broadcast_to([128, HPC, 1]))

            assert repeat <= unroll or repeat % unroll == 0
            rep_ctx = (tc.For_i(0, repeat // unroll, 1) if repeat > unroll
                       else contextlib.nullcontext())
            rep_ctx.__enter__()

            qts = [None] * PAIRS
            kts = [None] * PAIRS
            ons = [None] * PAIRS

            def alloc_qk(p):
                if qts[p] is None:
                    qk_dt = BF16 if qk16 else F32R
                    qts[p] = big.tile([128, S], qk_dt, tag="big", name=f"qt{p}")
                    kts[p] = big.tile([128, S], qk_dt, tag="big", name=f"kt{p}")

            def emit_qk_unit(p, c, which):
                """One projection unit: 6 matmuls + copyback for Q or K,
                pair p, sequence chunk c."""
                alloc_qk(p)
                w_sb, dst = (wq, qts[p]) if which == "q" else (wk, kts[p])
                ps = mm_ps.tile([128, QC], F32, tag="mm")
                if split_acc:
                    # two half-column chains interleaved in one bank: the
                    # consecutive matmuls alternate PSUM half-regions so an
                    # accumulate never immediately follows a write to the
                    # same addresses
                    hc = QC // 2
                    for e in range(EKT):
                        for half in range(2):
                            nc.tensor.matmul(
                                ps[:, half * hc:(half + 1) * hc],
                                w_sb[:, e, p * 128:(p + 1) * 128],
                                xts[e][:, c * QC + half * hc:
                                          c * QC + (half + 1) * hc],
                                start=(e == 0 and half == 0),
                                stop=(e == EKT - 1 and half == 1))
                else:
                    for e in range(EKT):
                        nc.tensor.matmul(
                            ps[:], w_sb[:, e, p * 128:(p + 1) * 128],
                            xts[e][:, c * QC:(c + 1) * QC],
                            start=(e == 0), stop=(e == EKT - 1))
                nc.vector.tensor_copy(dst[:, c * QC:(c + 1) * QC], ps[:])

            def emit_v_unit(st):
                """V for s-tile st, all 6 heads."""
                ps = mm_ps.tile([128, HPC * HD], F32, tag="mm")
                if split_acc:
                    hc = HPC * HD // 2
                    for e in range(EKT):
                        for half in range(2):
                            nc.tensor.matmul(
                                ps[:, half * hc:(half + 1) * hc],
                                xts[e][:, st * 128:(st + 1) * 128],
                                wv[:, e, half * hc:(half + 1) * hc],
                                start=(e == 0 and half == 0),
                                stop=(e == EKT - 1 and half == 1))
                else:
                    for e in range(EKT):
                        nc.tensor.matmul(
                            ps[:], xts[e][:, st * 128:(st + 1) * 128],
                            wv[:, e, :],
                            start=(e == 0), stop=(e == EKT - 1))
                nc.vector.tensor_copy(
                    v_sb[st][:, :, 0:HD],
                    ps[:].rearrange("p (h d) -> p h d", h=HPC))

            def emit_proj_unit(qc, et):
                """Output projection for q-chunk qc, one e-tile."""
                o_sb = nrm.tile([128, QC], F32, tag="o_sb", bufs=2)
                ps = mm_ps.tile([128, QC], F32, tag="mm")
                if split_acc:
                    hc = QC // 2
                    for p in range(PAIRS):
                        for half in range(2):
                            nc.tensor.matmul(
                                ps[:, half * hc:(half + 1) * hc],
                                wp[:, p, et * 128:(et + 1) * 128],
                                ons[p][:, qc * QC + half * hc:
                                          qc * QC + (half + 1) * hc],
                                start=(p == 0 and half == 0),
                                stop=(p == PAIRS - 1 and half == 1))
                else:
                    for p in range(PAIRS):
                        nc.tensor.matmul(
                            ps[:], wp[:, p, et * 128:(et + 1) * 128],
                            ons[p][:, qc * QC:(qc + 1) * QC],
                            start=(p == 0), stop=(p == PAIRS - 1))
                nc.vector.tensor_copy(o_sb[:], ps[:])
                nc.sync.dma_start(
                    o_d[et * 128:(et + 1) * 128, qc * QC:(qc + 1) * QC], o_sb[:])

            def emit_proj_partial(et):
                """Pairs 0-1 of the last q-chunk's projection, copied to
                SBUF so only pair 2's matmul remains after the tail."""
                qc = NQC - 1
                ps = mm_ps.tile([128, QC], F32, tag="mm")
                for p in range(2):
                    nc.tensor.matmul(
                        ps[:], wp[:, p, et * 128:(et + 1) * 128],
                        ons[p][:, qc * QC:(qc + 1) * QC],
                        start=(p == 0), stop=(p == 1))
                part = nrm.tile([128, QC], F32, tag="part", bufs=6)
                nc.vector.tensor_copy(part[:], ps[:])
                return part

            def emit_proj_final(et, part):
                qc = NQC - 1
                ps = mm_ps.tile([128, QC], F32, tag="mm")
                nc.tensor.matmul(
                    ps[:], wp[:, 2, et * 128:(et + 1) * 128],
                    ons[2][:, qc * QC:(qc + 1) * QC],
                    start=True, stop=True)
                o_sb = nrm.tile([128, QC], F32, tag="o_sb", bufs=2)
                nc.vector.tensor_add(o_sb[:], part[:], ps[:])
                nc.sync.dma_start(
                    o_d[et * 128:(et + 1) * 128, qc * QC:(qc + 1) * QC],
                    o_sb[:])

            def emit_attn_qc(p, qc, fillers):
                """One q-chunk (512) of attention for head pair p.

                `fillers` is a list of zero-arg emission callbacks (QKV or
                proj units for other pairs) spread across the kt loop so
                the PE/DVE queues never hold a long serial run while the
                scalar engine streams exps.
                """
                if av_qd:
                    # out in [q, d] layout: P^T tile is the stationary,
                    # V (+ ones col) streams 65 columns; denominator lands
                    # in column 64 of each q row.  Head h accumulates in
                    # PSUM bank h (av[:, h, :] is one 2KB bank).
                    av = av_ps.tile([128, 2, QC], F32, tag="av")
                else:
                    av = av_ps.tile([65, 2, QC], F32, tag="av")
                AV_LAG = lag  # emit AV(kt-LAG) after ST(kt) so a blocked AV
                #             (av WAR on the previous chunk's normalize)
                #             never starves the exp stream of fresh STs
                pts = {}

                def emit_av(kt):
                    pt = pts.pop(kt)
                    if "tinyav" in ablate:
                        for h in range(2):
                            if av_qd:
                                nc.tensor.matmul(
                                    av[:, h, 0:8],
                                    pt[:, h * QC:h * QC + 128],
                                    v_sb[kt][:, 2 * p + h, 0:8],
                                    start=(kt == 0), stop=(kt == SKT - 1))
                            else:
                                nc.tensor.matmul(
                                    av[:, h, 0:8], v_sb[kt][:, 2 * p + h, :],
                                    pt[:, h * QC:h * QC + 8],
                                    start=(kt == 0), stop=(kt == SKT - 1))
                        return
                    if av_qd:
                        # 4 q-subtile chains share PSUM bank h: one
                        # accumulation group per bank (2KB zero region) --
                        # start marks the whole bank pending-zero, so only
                        # the first chain starts and only the last stops.
                        for qt in range(4):
                            for h in range(2):
                                nc.tensor.matmul(
                                    av[:, h, qt * 65:qt * 65 + 65],
                                    pt[:, h * QC + qt * 128:
                                          h * QC + (qt + 1) * 128],
                                    v_sb[kt][:, 2 * p + h, :],
                                    start=(kt == 0 and qt == 0),
                                    stop=(kt == SKT - 1 and qt == 3))
                        return
                    for h in range(2):
                        nc.tensor.matmul(
                            av[:, h, :], v_sb[kt][:, 2 * p + h, :],
                            pt[:, h * QC:(h + 1) * QC],
                            start=(kt == 0), stop=(kt == SKT - 1))

                st2 = pt2 = None
                for kt in range(SKT):
                    if exp2:
                        # one 4-bank st tile and one exp per TWO kt: halves
                        # the ACT instruction count (each activation carries
                        # ~170 cyc of access-latency overhead)
                        if kt % 2 == 0:
                            st2 = st_ps.tile([128, 2, 2 * QC], F32,
                                             tag="st")
                            pt2 = ptp.tile([128, 2, 2 * QC], BF16,
                                           tag="pt")
                        st = st2[:, kt % 2, :]
                    else:
                        st = st_ps.tile([128, 2 * QC], F32, tag="st")
                    # the second (rows 64-127) score matmul gets a priority
                    # boost so the scheduler keeps the row-tiled pair
                    # adjacent in the PE queue -- adjacent 64-row matmuls
                    # co-stream on disjoint row groups (2x throughput);
                    # an earlier-emitted AV becoming ready mid-pair would
                    # otherwise be heap-popped between them.
                    # st_hi > 0 additionally lifts BOTH score matmuls above
                    # every filler/AV in the ready-heap (readiness still
                    # gates them via the st_ps WAR), so a 6-matmul filler
                    # unit never starves the ACT exp feed.
                    if st_hi:
                        import contextlib as _cl
                        hi1 = tc.high_priority(offset=st_hi)
                        hi2 = tc.high_priority(offset=st_hi)
                    else:
                        import contextlib as _cl
                        hi1 = _cl.nullcontext()
                        hi2 = tc.high_priority(offset=200)
                    with hi1:
                        nc.tensor.matmul(
                            st[:, 0:QC],
                            kts[p][0:64, kt * 128:(kt + 1) * 128],
                            qts[p][0:64, qc * QC:(qc + 1) * QC],
                            start=True, stop=True)
                    with hi2:
                        nc.tensor.matmul(
                            st[:, QC:2 * QC],
                            kts[p][64:128, kt * 128:(kt + 1) * 128],
                            qts[p][64:128, qc * QC:(qc + 1) * QC],
                            start=True, stop=True)
                    if exp2:
                        if kt % 2 == 1:
                            if "tinyexp" in ablate:
                                nc.scalar.activation(
                                    pt2[:, 0, 0:8], st2[:, 0, 0:8], Exp,
                                    scale=float(HD) ** -0.5)
                            else:
                                nc.scalar.activation(
                                    pt2[:], st2[:], Exp,
                                    scale=float(HD) ** -0.5)
                            pts[kt - 1] = pt2[:, 0, :]
                            pts[kt] = pt2[:, 1, :]
                    else:
                        pt = ptp.tile([128, 2 * QC], BF16, tag="pt")
                        if "tinyexp" in ablate:
                            nc.scalar.activation(pt[:, 0:8], st[:, 0:8],
                                                 Exp,
                                                 scale=float(HD) ** -0.5)
                        else:
                            nc.scalar.activation(pt[:], st[:], Exp,
                                                 scale=float(HD) ** -0.5)
                        pts[kt] = pt
                    if kt >= AV_LAG:
                        emit_av(kt - AV_LAG)
                    if kt < len(fillers):
                        fillers[kt]()
                for kt in range(SKT - AV_LAG, SKT):
                    emit_av(kt)
                if ons[p] is None:
                    ons[p] = big.tile([128, S], BF16 if proj16 else F32R,
                                      tag="big", name=f"on{p}")
                if av_qd:
                    # [q, d] layout: per-partition denominator in col 64.
                    # Normalize with a free-dim-broadcast reciprocal, then
                    # transpose 128x128 (two heads stacked) back to [hd, q]
                    # on the PE for the output projection.
                    on_p = ons[p]

                    def tail_norm():
                        av_sb = nrm.tile([128, 2, 4, 65], F32, tag="av_sb",
                                         bufs=2)
                        nc.vector.tensor_copy(
                            av_sb[:],
                            av[:, :, 0:260].rearrange(
                                "p h (q d) -> p h q d", d=65))
                        recip = nrm.tile([128, 2, 4, 1], F32R, tag="recip",
                                         bufs=1)
                        with nc.allow_low_precision(reason="f32r recip"):
                            nc.vector.reciprocal(recip[:],
                                                 av_sb[:, :, :, 64:65])
                        # qt-major so each transpose input slice nat[:, qt]
                        # is one contiguous 128-wide free dim (walrus
                        # allows only one free dim on the stationary AP)
                        nat = nrm.tile([128, 4, 2, HD], BF16, tag="nat",
                                       bufs=2)
                        nc.vector.tensor_mul(
                            nat[:].rearrange("p qt h d -> p h qt d"),
                            av_sb[:, :, :, 0:HD],
                            recip[:].broadcast_to([128, 2, 4, HD]))
                        return nat

                    def tail_tp(nat, qts_):
                        for qt in qts_:
                            tp = mm_ps.tile([128, 128], BF16, tag="mm")
                            nc.tensor.transpose(tp[:], nat[:, qt, :, :],
                                                ident[:])
                            nc.vector.tensor_copy(
                                on_p[:, qc * QC + qt * 128:
                                        qc * QC + (qt + 1) * 128], tp[:])

                    if tp_defer:
                        st_ = {}

                        def t0():
                            st_["nat"] = tail_norm()

                        return [t0,
                                lambda: tail_tp(st_["nat"], (0, 1)),
                                lambda: tail_tp(st_["nat"], (2, 3))]
                    nat = tail_norm()
                    tail_tp(nat, (0, 1, 2, 3))
                    return None
                # single copy frees the av accumulator; the rest of the
                # normalize chain runs off-PSUM without blocking the next
                # chunk's AV matmuls
                av_sb = nrm.tile([65, 2, QC], F32, tag="av_sb", bufs=2)
                nc.vector.tensor_copy(av_sb[:], av[:])
                recip = nrm.tile([1, 2, QC], F32R, tag="recip", bufs=1)
                with nc.allow_low_precision(reason="f32r recip for PE bcast"):
                    nc.vector.reciprocal(recip[0:1, :, :], av_sb[64:65, :, :])
                for h in range(2):
                    if dma_bc:
                        # partition-broadcast the [1, QC] reciprocal row to
                        # 64 partitions on the DMA engine (stride-0 source)
                        # instead of a ones-stationary PE matmul
                        bc_sb = nrm.tile([64, QC], F32R, tag="bc_sb", bufs=2)
                        nc.sync.dma_start(
                            bc_sb[:],
                            recip[0:1, h, :].broadcast_to([64, QC]))
                        bc = bc_sb
                    else:
                        bc = mm_ps.tile([64, QC], F32, tag="mm")
                        nc.tensor.matmul(bc[:], ones_r[0:1, :],
                                         recip[0:1, h, :],
                                         start=True, stop=True)
                    nc.vector.tensor_mul(
                        ons[p][h * 64:(h + 1) * 64, qc * QC:(qc + 1) * QC],
                        av_sb[0:64, h, :], bc[:])

            # ---- emission schedule ----
            # Correctness constraints on emission order (Tile serializes
            # reads after writes in program order):
            #  - K(p, c) is read by attn(p, qc, kt=4c..4c+3) for EVERY qc,
            #    so all 4 K chunks must be emitted before (or early inside
            #    via fillers) pair p's first q-chunk.
            #  - Q(p, c) is read only by attn(p, qc=c).
            #  - V(s) is read by AV(kt=s), emitted at loop slot s+AV_LAG.
            # Everything not needed immediately rides as a filler unit so
            # the scalar engine's exp stream starts ~5us in.
            def emit_body():
                qts[:] = [None] * PAIRS
                kts[:] = [None] * PAIRS
                ons[:] = [None] * PAIRS

                emit_qk_unit(0, 0, "q")
                emit_qk_unit(0, 0, "k")
                emit_qk_unit(0, 1, "q")
                for s in range(3):
                    emit_v_unit(s)

                qk = lambda p, c, w: (lambda: emit_qk_unit(p, c, w))
                vu = lambda st: (lambda: emit_v_unit(st))
                pj = lambda q, e: (lambda: emit_proj_unit(q, e))

                # Fillers ride in the PE slack of the ACT-bound kt loops.
                # Ordering constraints: K(p, c) before pair p's first chunk
                # reads key-tile 4c; Q(p, c) before chunk (p, qc=c); V(s) no
                # later than slot s + AV_LAG - 1 of (0, 0).
                if sched2:
                    # spread Q/K units across the otherwise filler-free
                    # mid-chunks so the PE never runs a long ST+AV-only
                    # stretch (where it outpaces the ACT exp stream and
                    # idles on st_ps buffer recycling).  Constraints:
                    # K(p, c) before pair p's first chunk, Q(p, c) before
                    # chunk (p, qc=c), V(s) in (0, 0), proj(qc) after
                    # chunk (2, qc)'s tail.
                    filler_sched = {
                        (0, 0): [qk(0, 1, "k"), qk(0, 2, "k"),
                                 qk(0, 3, "k")]
                                + [vu(s) for s in range(3, 16)],
                        (0, 1): [qk(0, 2, "q"), qk(1, 0, "k"),
                                 qk(1, 1, "k")],
                        (0, 2): [qk(0, 3, "q"), qk(1, 2, "k"),
                                 qk(1, 3, "k"), qk(1, 0, "q")],
                        (0, 3): [qk(1, 1, "q")],
                        (1, 0): [qk(1, 2, "q"), qk(2, 0, "k"),
                                 qk(2, 0, "q")],
                        (1, 1): [qk(1, 3, "q"), qk(2, 1, "k")],
                        (1, 2): [qk(2, 2, "k")],
                        (1, 3): [qk(2, 3, "k")],
                        (2, 0): [qk(2, 1, "q")],
                        (2, 1): [qk(2, 2, "q")]
                                + [pj(0, e) for e in range(EMBED // 128)],
                        (2, 2): [qk(2, 3, "q")]
                                + [pj(1, e) for e in range(EMBED // 128)],
                        (2, 3): [pj(2, e) for e in range(EMBED // 128)],
                    }
                else:
                    filler_sched = {
                        (0, 0): [qk(0, 1, "k"), qk(0, 2, "k"), qk(0, 3, "k")]
                                + [vu(s) for s in range(3, 16)],
                        (0, 1): [qk(0, 2, "q"), qk(0, 3, "q"), qk(1, 0, "q"),
                                 qk(1, 0, "k"), qk(1, 1, "k"), qk(1, 2, "k"),
                                 qk(1, 3, "k")],
                        (0, 2): [qk(1, 1, "q"), qk(1, 2, "q"), qk(1, 3, "q")],
                        (1, 0): [qk(2, 0, "q"), qk(2, 0, "k"), qk(2, 1, "k"),
                                 qk(2, 2, "k"), qk(2, 3, "k")],
                        (1, 1): [qk(2, 1, "q"), qk(2, 2, "q"), qk(2, 3, "q")],
                        (2, 1): [pj(0, e) for e in range(EMBED // 128)],
                        (2, 2): [pj(1, e) for e in range(EMBED // 128)],
                        (2, 3): [pj(2, e) for e in range(EMBED // 128)],
                    }
                parts = {}
                if tail_proj:
                    def ppart(et):
                        def f():
                            parts[et] = emit_proj_partial(et)
                        return f
                    filler_sched[(2, NQC - 1)] = (
                        filler_sched.get((2, NQC - 1), [])
                        + [ppart(et) for et in range(EMBED // 128)])

                pending = []
                for p in range(PAIRS):
                    for qc in range(NQC):
                        fillers = pending + filler_sched.get((p, qc), [])
                        assert len(fillers) <= SKT
                        pending = emit_attn_qc(p, qc, fillers) or []
                for t in pending:
                    t()
                for et in range(EMBED // 128):
                    if tail_proj:
                        emit_proj_final(et, parts[et])
                    else:
                        emit_proj_unit(NQC - 1, et)

            for _ in range(min(unroll, repeat)):
                emit_body()

            rep_ctx.__exit__(None, None, None)

    nc.compile()
    return nc


def _get_nc():
    if "nc" not in _CACHE:
        _CACHE["nc"] = _build()
    return _CACHE["nc"]


PROJ16 = True    # ship W_proj as bfloat16 too


def in_dt16():
    import ml_dtypes
    return ml_dtypes.bfloat16


def _shard(x, W_qkv, W_proj):
    if BF16_IN:
        in_dt = in_dt16()
    else:
        in_dt = np.float32
    ins = []
    for c in range(N_CORES):
        b, g = divmod(c, 2)
        cols = slice(g * HPC * HD, (g + 1) * HPC * HD)
        ins.append({
            "xt": np.ascontiguousarray(x[b].T).astype(in_dt),
            "wq": np.ascontiguousarray(
                W_qkv[:, 0 * EMBED:1 * EMBED][:, cols]).astype(in_dt),
            "wk": np.ascontiguousarray(
                W_qkv[:, 1 * EMBED:2 * EMBED][:, cols]).astype(in_dt),
            "wv": np.ascontiguousarray(
                W_qkv[:, 2 * EMBED:3 * EMBED][:, cols]).astype(in_dt),
            "wp": np.ascontiguousarray(W_proj[cols, :]).astype(
                np.float32 if not PROJ16 else in_dt16()),
        })
    return ins


def kernel(x, W_qkv, W_proj, b_proj):
    from concourse.bass_utils import run_bass_kernel_spmd
    global LAST_RESULTS

    x = np.asarray(x, dtype=np.float32)
    W_qkv = np.asarray(W_qkv, dtype=np.float32)
    W_proj = np.asarray(W_proj, dtype=np.float32)
    b_proj = np.asarray(b_proj, dtype=np.float32)

    ins = _shard(x, W_qkv, W_proj)

    nc = _get_nc()
    res = run_bass_kernel_spmd(nc, ins, core_ids=list(range(N_CORES)))
    LAST_RESULTS = res

    out = np.empty((B, S, EMBED), np.float32)
    for b in range(B):
        acc = res.results[2 * b]["o"] + res.results[2 * b + 1]["o"]
        out[b] = acc.T + b_proj
    return out

